# revision 3
# baseline (speedup 1.0000x reference)
"""GCN message-passing kernel for 8 Trainium2 NeuronCores.

Model (PyG GCNConv x3 + MLP head + softmax):
    A01 = adjacency + self loops (unit weights), deg = in-degree over A01
    conv(H, W) = D^-1/2 A01 D^-1/2 (H @ W)
    h = x; h = leaky(conv(h, Wg_l)) x3
    y = softmax(leaky(leaky(h @ Wfc1) @ Wfc2))

Key algebraic rewrite: leaky_relu is positively homogeneous, so the
D^-1/2 factors can be pulled out of every leaky() and folded into the
per-layer "message table" G_l:
    G_1 = D^-1/2 (x @ Wg0)
    Ht_{l+1} = leaky(A01 @ G_l)            (pure 0/1 segment-sum!)
    G_{l+1} = D^-1 (Ht_{l+1} @ Wg_l)
    final: z = D^-1/2 leaky(leaky(Ht_4 @ Wfc1) @ Wfc2), out = softmax(z)

Sharding: destination nodes are split into 8 contiguous blocks of 6250
(padded to 6272 = 49 windows of 128). Each layer: every core computes its
G shard (matmul + per-row scale), an AllGather builds the full G table in
DRAM, then each core gathers source rows for its edges with
dma_gather(transpose=True) (int16 indices sorted ascending per bucket for
DRAM locality; table split in two <32768-row halves; ragged per-window
chunk caps = max over cores). The feature-major gather output is
PE-transposed back to edge-major 4 chunks per PSUM bank, moved to SBUF by
one wide DVE copy, and accumulated  msg^T @ onehot  into a PSUM window on
the TensorEngine (one-hot built on DVE via is_equal vs an iota row).
Self-loop messages skip the gather entirely: one identity-rhs matmul adds
the window's own G rows (read back contiguously from the local shard).
The flush produces the next layer's activations already transposed
(feat x rows), which is exactly the lhsT layout the next matmul needs.
"""

import numpy as np

P = 128
N_CORES = 8


def _gw(NW):
    """Windows per gather group."""
    return 7 if NW % 7 == 0 else 1


# --------------------------------------------------------------------------
# Host-side preprocessing: shard edges by destination, pad to fixed chunk
# counts (SPMD requires an identical instruction stream on all cores).
# --------------------------------------------------------------------------
def _preprocess(x, edge_index):
    N, D = x.shape
    assert D == P
    NL = N // N_CORES                      # real nodes per core
    NW = (NL + P - 1) // P                 # windows per core
    NLP = NW * P                           # padded nodes per core
    NGP = N_CORES * NLP                    # padded global nodes
    # A/B source-table split at the per-core window midpoint: table A holds
    # every core's windows [0, SPLIT_W), table B the rest.  Both stay under
    # the 32768-row int16 gather limit, and the A-half AllGather can fire
    # as soon as each core finishes producing its first SPLIT_W windows,
    # overlapping the B-half production and the B AllGather with the
    # A-half gathers.
    SPLIT_W = (NW + 1) // 2
    SPLIT = SPLIT_W * P

    src = np.asarray(edge_index[0], dtype=np.int64)
    dst = np.asarray(edge_index[1], dtype=np.int64)

    # in-degree INCLUDING the self loop (GCNConv adds A+I); the self-loop
    # message itself is applied on-device via an identity matmul over the
    # local G rows, so loop edges are NOT bucketed.
    deg = (np.bincount(dst, minlength=N) + 1).astype(np.float32)

    sowner = src // NL
    lid_s = src - sowner * NL              # source local id on its core

    owner = dst // NL                      # destination owner core
    lid = dst - owner * NL                 # local dest id on that core
    w = lid // P                           # window
    dr = (lid % P).astype(np.float32)      # one-hot row within window
    half = (lid_s >= SPLIT).astype(np.int64)
    srel = np.where(half == 1,
                    sowner * (NLP - SPLIT) + (lid_s - SPLIT),
                    sowner * SPLIT + lid_s)  # row within table A/B
    assert N_CORES * SPLIT <= 32768 and N_CORES * (NLP - SPLIT) <= 32768

    # bucket key: (core, half, window); sort by srel within each bucket so
    # the gather walks ascending rows (DRAM page locality)
    key = ((owner * 2 + half) * NW + w)
    nbuckets = N_CORES * 2 * NW
    order = np.lexsort((srel, key))
    key_s = key[order]
    srel_s = srel[order]
    dr_s = dr[order]

    counts = np.bincount(key_s, minlength=nbuckets).reshape(N_CORES, 2, NW)
    # ragged per-window chunk caps (max over cores, shared SPMD stream)
    capl = np.ceil(counts[:, 0, :].max(axis=0) / P).astype(np.int64)  # [NW]
    caph = np.ceil(counts[:, 1, :].max(axis=0) / P).astype(np.int64)
    capl = np.maximum(capl, 1)   # keep streams non-empty (small graphs)
    caph = np.maximum(caph, 1)
    CAPW = np.stack([capl, caph], axis=0) * P          # [2, NW] slots
    TL = int(CAPW[0].sum())                            # lo slots per core
    TH = int(CAPW[1].sum())

    # per-(core, half, window) slot base inside that core's lo/hi stream
    wbase = np.zeros((2, NW), dtype=np.int64)
    wbase[0, 1:] = np.cumsum(CAPW[0])[:-1]
    wbase[1, 1:] = np.cumsum(CAPW[1])[:-1]

    start = np.zeros(nbuckets, dtype=np.int64)
    start[1:] = np.cumsum(counts.reshape(-1))[:-1]
    within = np.arange(len(key_s)) - start[key_s]
    h_s = (key_s // NW) % 2
    w_s = key_s % NW
    c_s = key_s // (2 * NW)
    dest = c_s * (TL + TH) + np.where(h_s == 0, 0, TL) \
        + wbase[h_s, w_s] + within

    total_cap = N_CORES * (TL + TH)
    idx_flat = np.zeros(total_cap, dtype=np.int16)
    dr_flat = np.full(total_cap, 200.0, dtype=np.float32)
    idx_flat[dest] = srel_s.astype(np.int16)
    dr_flat[dest] = dr_s

    per_core = []
    for c in range(N_CORES):
        seg_i = idx_flat[c * (TL + TH):(c + 1) * (TL + TH)]
        seg_d = dr_flat[c * (TL + TH):(c + 1) * (TL + TH)]
        ilo, ihi = seg_i[:TL], seg_i[TL:]
        dlo, dhi = seg_d[:TL], seg_d[TL:]
        per_core.append((ilo, ihi, dlo, dhi))

    meta = dict(N=N, NL=NL, NW=NW, NLP=NLP, NGP=NGP, SPLIT_W=SPLIT_W,
                capl=[int(v) for v in capl], caph=[int(v) for v in caph],
                TL=TL, TH=TH, deg=deg)
    return per_core, meta


def _wrap_idx_ragged(flat, gbounds):
    """flat: [T] int16 slot stream (window-major, ragged caps).
    gbounds: slot offsets of each gather-group boundary (len ngrp+1).
    Returns [128, T/16] int16 in dma_gather's wrapped layout: within each
    group block, logical index j lives at [j % 16, j // 16], replicated
    8x across the 128 partitions."""
    cols = []
    for g in range(len(gbounds) - 1):
        block = flat[gbounds[g]:gbounds[g + 1]]
        m = block.reshape(-1, 16).T
        cols.append(np.tile(m, (8, 1)))
    return np.ascontiguousarray(np.concatenate(cols, axis=1))


def _group_bounds(cap, GW):
    """Slot/chunk offsets per gather group given per-window caps."""
    NW = len(cap)
    ngrp = NW // GW
    sb = [0]
    kb = [0]
    for g in range(ngrp):
        sb.append(sb[-1] + sum(cap[g * GW:(g + 1) * GW]) * P)
        kb.append(kb[-1] + sum(cap[g * GW:(g + 1) * GW]))
    return sb, kb


def _build_core_inputs(x, Ws, per_core, meta):
    """Build the per-core device input dict."""
    N, NL, NW, NLP = meta["N"], meta["NL"], meta["NW"], meta["NLP"]
    capl, caph = meta["capl"], meta["caph"]
    deg = meta["deg"]
    GW = _gw(NW)
    Wg0, Wg1, Wg2, Wfc1, Wfc2 = Ws

    iota = np.tile(np.arange(P, dtype=np.float32), (P, 1))
    ident = np.eye(P, dtype=np.float32)
    # Wfc2 [256, 2] -> [128, 4]: cols 0:2 first half of u, 2:4 second half
    Wfc2p = np.concatenate([Wfc2[:P, :], Wfc2[P:, :]], axis=1)
    Wfc2p = np.ascontiguousarray(Wfc2p, dtype=np.float32)

    sbl, _ = _group_bounds(capl, GW)
    sbh, _ = _group_bounds(caph, GW)

    def dstrel(dflat, cap):
        # [128, sum(cap)]: col = chunk (window-major), row p = edge slot
        cols = []
        o = 0
        for w_ in range(NW):
            n = cap[w_] * P
            cols.append(dflat[o:o + n].reshape(cap[w_], P).T)
            o += n
        return np.ascontiguousarray(np.concatenate(cols, axis=1))

    in_maps = []
    for c in range(N_CORES):
        ilo, ihi, dlo, dhi = per_core[c]
        xs = np.zeros((NLP, P), dtype=np.float32)
        xs[:NL] = x[c * NL:(c + 1) * NL]
        x_t = np.ascontiguousarray(xs.T)                   # [128, NLP]

        degp = np.ones(NLP, dtype=np.float32)
        degp[:NL] = deg[c * NL:(c + 1) * NL]
        deg_t = np.ascontiguousarray(degp.reshape(NW, P).T)  # [128, NW]

        in_maps.append({
            "x_t": x_t,
            "deg_t": deg_t,
            "idx_lo": _wrap_idx_ragged(ilo, sbl),
            "dstrel_lo": dstrel(dlo, capl),
            "idx_hi": _wrap_idx_ragged(ihi, sbh),
            "dstrel_hi": dstrel(dhi, caph),
            "iota": iota,
            "ident": ident,
            "Wg0": np.ascontiguousarray(Wg0, dtype=np.float32),
            "Wg1": np.ascontiguousarray(Wg1, dtype=np.float32),
            "Wg2": np.ascontiguousarray(Wg2, dtype=np.float32),
            "Wfc1": np.ascontiguousarray(Wfc1, dtype=np.float32),
            "Wfc2p": Wfc2p,
        })
    return in_maps


# --------------------------------------------------------------------------
# Device program
# --------------------------------------------------------------------------
def _build_bass(meta, mock_cc=False, opts=None, reps=1):
    opts = opts or {}
    from concourse import bass, bacc, mybir
    import concourse.tile as tile

    NW, NLP, NGP = meta["NW"], meta["NLP"], meta["NGP"]
    SPLIT_W = meta["SPLIT_W"]
    SPLIT = SPLIT_W * P
    NBW = NW - SPLIT_W                     # B-half windows (may be 0)
    NA = N_CORES * SPLIT                   # table A rows
    NB = N_CORES * (NLP - SPLIT)           # table B rows
    capl, caph = meta["capl"], meta["caph"]
    TL, TH = meta["TL"], meta["TH"]
    GW = _gw(NW)
    NGRP = NW // GW
    sbl, kbl = _group_bounds(capl, GW)     # slot / chunk offsets per group
    sbh, kbh = _group_bounds(caph, GW)
    ckl = np.concatenate([[0], np.cumsum(capl)]).astype(int)  # per window
    ckh = np.concatenate([[0], np.cumsum(caph)]).astype(int)
    GLMAXL = max(sbl[g + 1] - sbl[g] for g in range(NGRP))
    GLMAXH = max(sbh[g + 1] - sbh[g] for g in range(NGRP))
    KMAXL = max(kbl[g + 1] - kbl[g] for g in range(NGRP))
    KMAXH = max(kbh[g + 1] - kbh[g] for g in range(NGRP))
    f32 = mybir.dt.float32
    bf16 = mybir.dt.bfloat16
    i16 = mybir.dt.int16
    ALL = [list(range(N_CORES))]

    nc = bacc.Bacc("TRN2", target_bir_lowering=False, debug=False,
                   num_devices=N_CORES)

    x_t_d = nc.dram_tensor("x_t", [P, NLP], f32, kind="ExternalInput")
    deg_d = nc.dram_tensor("deg_t", [P, NW], f32, kind="ExternalInput")
    ilo_d = nc.dram_tensor("idx_lo", [P, TL // 16], i16, kind="ExternalInput")
    drl_d = nc.dram_tensor("dstrel_lo", [P, TL // P], f32,
                           kind="ExternalInput")
    ihi_d = nc.dram_tensor("idx_hi", [P, TH // 16], i16, kind="ExternalInput")
    drh_d = nc.dram_tensor("dstrel_hi", [P, TH // P], f32,
                           kind="ExternalInput")
    iota_d = nc.dram_tensor("iota", [P, P], f32, kind="ExternalInput")
    ident_d = nc.dram_tensor("ident", [P, P], f32, kind="ExternalInput")
    wg_d = [nc.dram_tensor(f"Wg{i}", [P, P], f32, kind="ExternalInput")
            for i in range(3)]
    wfc1_d = nc.dram_tensor("Wfc1", [P, 256], f32, kind="ExternalInput")
    wfc2_d = nc.dram_tensor("Wfc2p", [P, 4], f32, kind="ExternalInput")
    out_d = nc.dram_tensor("out", [NLP, 2], f32, kind="ExternalOutput")

    with tile.TileContext(nc) as tc:
        with (
            tc.tile_pool(name="const", bufs=1) as cpool,
            tc.tile_pool(name="msg", bufs=2) as mpool,
            tc.tile_pool(name="oh", bufs=2) as ohpool,
            tc.tile_pool(name="work", bufs=3) as wpool,
            tc.tile_pool(name="acc", bufs=3, space="PSUM") as ppool,
            tc.tile_pool(name="accy", bufs=1, space="PSUM") as p2pool,
            tc.tile_pool(name="tpsum", bufs=4, space="PSUM") as tpool,
            tc.tile_pool(name="msgs", bufs=8) as mspool,
            tc.tile_pool(name="dram", bufs=1, space="DRAM") as dpool,
        ):
            # ---- constants / casts ----
            T_a = cpool.tile([P, NLP], bf16, name="T_a")
            nc.gpsimd.dma_start(out=T_a[:], in_=x_t_d[:])   # f32->bf16 cast
            T_b = cpool.tile([P, NLP], bf16, name="T_b")

            iota_sb = cpool.tile([P, P], bf16, name="iota_sb")
            nc.gpsimd.dma_start(out=iota_sb[:], in_=iota_d[:])
            ident_sb = cpool.tile([P, P], bf16, name="ident_sb")
            nc.gpsimd.dma_start(out=ident_sb[:], in_=ident_d[:])
            wg_sb = []
            for i in range(3):
                t = cpool.tile([P, P], bf16, name=f"wg_sb{i}")
                nc.gpsimd.dma_start(out=t[:], in_=wg_d[i][:])
                wg_sb.append(t)
            wfc1_sb = cpool.tile([P, 256], bf16, name="wfc1_sb")
            nc.gpsimd.dma_start(out=wfc1_sb[:], in_=wfc1_d[:])
            wfc2_sb = cpool.tile([P, 4], bf16, name="wfc2_sb")
            nc.gpsimd.dma_start(out=wfc2_sb[:], in_=wfc2_d[:])
            drl_sb = cpool.tile([P, TL // P], bf16, name="drl_sb")
            nc.gpsimd.dma_start(out=drl_sb[:], in_=drl_d[:])  # f32->bf16
            ilo_sb = cpool.tile([P, TL // 16], i16, name="ilo_sb")
            nc.sync.dma_start(out=ilo_sb[:], in_=ilo_d[:])
            drh_sb = cpool.tile([P, TH // P], bf16, name="drh_sb")
            nc.gpsimd.dma_start(out=drh_sb[:], in_=drh_d[:])
            ihi_sb = cpool.tile([P, TH // 16], i16, name="ihi_sb")
            nc.sync.dma_start(out=ihi_sb[:], in_=ihi_d[:])

            deg_sb = cpool.tile([P, NW], f32, name="deg_sb")
            nc.sync.dma_start(out=deg_sb[:], in_=deg_d[:])
            invdeg = cpool.tile([P, NW], f32, name="invdeg")
            nc.vector.reciprocal(invdeg[:], deg_sb[:])
            dinv = cpool.tile([P, NW], f32, name="dinv")
            nc.scalar.sqrt(dinv[:], invdeg[:])

            # NB: collective outputs in Local addr space — Shared
            # scratchpad DMA reads measured ~3x slower on the gather path.
            # Separate A/B local shards + tables give the Tile scheduler
            # precise deps: AG_A fires after the first SPLIT_W windows of
            # production and overlaps the rest; AG_B overlaps the A-half
            # gathers of the scatter phase.
            gfA = [dpool.tile([NA, P], bf16, name=f"gfA{i}") for i in range(3)]
            gfB = ([dpool.tile([NB, P], bf16, name=f"gfB{i}") for i in range(3)]
                   if NBW else None)
            glocA = [dpool.tile([SPLIT, P], bf16, name=f"glocA{i}")
                     for i in range(3)]
            glocB = ([dpool.tile([NLP - SPLIT, P], bf16, name=f"glocB{i}")
                      for i in range(3)] if NBW else None)

            Copy = mybir.ActivationFunctionType.Copy

            def g_production(l, Tsrc):
                scale = dinv if l == 0 else invdeg
                for w in range(NW):
                    ps = ppool.tile([P, P], f32, tag="acc", name="psg")
                    nc.tensor.matmul(ps[:], lhsT=Tsrc[:, w * P:(w + 1) * P],
                                     rhs=wg_sb[l][:], start=True, stop=True)
                    gw_t = wpool.tile([P, P], bf16, tag="gw", name="gw_t")
                    nc.scalar.activation(gw_t[:], ps[:], Copy,
                                         bias=0.0, scale=scale[:, w:w + 1])
                    if w < SPLIT_W:
                        nc.sync.dma_start(
                            out=glocA[l][w * P:(w + 1) * P, :], in_=gw_t[:])
                    else:
                        wb = w - SPLIT_W
                        nc.sync.dma_start(
                            out=glocB[l][wb * P:(wb + 1) * P, :], in_=gw_t[:])
                    if w == SPLIT_W - 1:
                        if mock_cc:
                            for c in range(N_CORES):
                                nc.sync.dma_start(
                                    out=gfA[l][c * SPLIT:(c + 1) * SPLIT, :],
                                    in_=glocA[l][:])
                        else:
                            nc.gpsimd.collective_compute(
                                "AllGather", mybir.AluOpType.bypass,
                                replica_groups=ALL,
                                ins=[glocA[l][:]], outs=[gfA[l][:]])
                if NBW:
                    if mock_cc:
                        nsh = NLP - SPLIT
                        for c in range(N_CORES):
                            nc.sync.dma_start(
                                out=gfB[l][c * nsh:(c + 1) * nsh, :],
                                in_=glocB[l][:])
                    else:
                        nc.gpsimd.collective_compute(
                            "AllGather", mybir.AluOpType.bypass,
                            replica_groups=ALL,
                            ins=[glocB[l][:]], outs=[gfB[l][:]])

            def leaky_into(dst_ap, ps):
                t = wpool.tile([P, dst_ap.shape[-1]], f32, tag="lk", name="lkt")
                nc.scalar.activation(t[:], ps[:], Copy, bias=0.0, scale=0.01)
                nc.vector.tensor_tensor(out=dst_ap, in0=ps[:], in1=t[:],
                                        op=mybir.AluOpType.max)


            def scatter_tr(l, Tdst):
                """Transpose-mode gather (dma_gather(transpose=True)) returns
                messages feature-major [128f, n_idxs]; each 128-edge chunk is
                PE-transposed back to edge-major via the identity trick,
                staged through PSUM(bf16) -> SBUF (DVE copy), then accumulated
                with the one-hot matmul.  Chunk counts are ragged per window;
                the self-loop message is injected exactly via an identity-rhs
                matmul over the core's own G rows (no gather slots)."""
                glo_ap = gfA[l][:]
                # degenerate graphs (single window) still issue B gathers
                # with idx 0 and all-dead one-hot columns
                ghi_ap = gfB[l][:] if NBW else gfA[l][0:P, :]

                # The Pool engine runs gathers in emission order, and every
                # B-table gather waits on AG_B.  Emit lo(g+1) BEFORE hi(g)
                # so A-side gathers keep streaming while AG_B is in flight.
                mtiles = {}

                def issue_lo(g):
                    GLLg = sbl[g + 1] - sbl[g]
                    t = mpool.tile([P, GLMAXL], bf16, tag="mlo", name="mlo")
                    if not opts.get("skip_gather"):
                        nc.gpsimd.dma_gather(
                            out_ap=t[:, 0:GLLg].rearrange("p (c e) -> p c e",
                                                          c=1),
                            in_ap=glo_ap,
                            idxs_ap=ilo_sb[:, sbl[g] // 16:sbl[g + 1] // 16],
                            num_idxs=GLLg, num_idxs_reg=GLLg, elem_size=P,
                            transpose=True, single_packet=False)
                    mtiles[("lo", g)] = t

                def issue_hi(g):
                    GLHg = sbh[g + 1] - sbh[g]
                    t = mpool.tile([P, GLMAXH], bf16, tag="mhi", name="mhi")
                    if not opts.get("skip_gather"):
                        nc.gpsimd.dma_gather(
                            out_ap=t[:, 0:GLHg].rearrange("p (c e) -> p c e",
                                                          c=1),
                            in_ap=ghi_ap,
                            idxs_ap=ihi_sb[:, sbh[g] // 16:sbh[g + 1] // 16],
                            num_idxs=GLHg, num_idxs_reg=GLHg, elem_size=P,
                            transpose=True, single_packet=False)
                    mtiles[("hi", g)] = t

                issue_lo(0)
                if NGRP > 1:
                    issue_lo(1)
                issue_hi(0)
                for g in range(NGRP):
                    KLg = kbl[g + 1] - kbl[g]
                    KHg = kbh[g + 1] - kbh[g]
                    mlo = mtiles.pop(("lo", g))
                    mhi = mtiles.pop(("hi", g))
                    ohb_lo = ohpool.tile([P, KMAXL * P], bf16, tag="ohlo",
                                         name="ohb_lo")
                    ohb_hi = ohpool.tile([P, KMAXH * P], bf16, tag="ohhi",
                                         name="ohb_hi")
                    if not opts.get("skip_onehot"):
                        nc.vector.tensor_tensor(
                            out=ohb_lo[:, 0:KLg * P].rearrange(
                                "p (k r) -> p k r", r=P),
                            in0=iota_sb[:].unsqueeze(1)
                                .to_broadcast([P, KLg, P]),
                            in1=drl_sb[:, kbl[g]:kbl[g + 1]].unsqueeze(2)
                                .to_broadcast([P, KLg, P]),
                            op=mybir.AluOpType.is_equal)
                        nc.vector.tensor_tensor(
                            out=ohb_hi[:, 0:KHg * P].rearrange(
                                "p (k r) -> p k r", r=P),
                            in0=iota_sb[:].unsqueeze(1)
                                .to_broadcast([P, KHg, P]),
                            in1=drh_sb[:, kbh[g]:kbh[g + 1]].unsqueeze(2)
                                .to_broadcast([P, KHg, P]),
                            op=mybir.AluOpType.is_equal)
                    if opts.get("skip_matmul"):
                        # timing-only variant: consume gathers/onehots with
                        # cheap DVE reduces so nothing dead-codes
                        X_ = mybir.AxisListType.X
                        tiles = []
                        if not opts.get("skip_gather"):
                            tiles += [mlo[:, 0:P], mhi[:, 0:P]]
                        if not opts.get("skip_onehot"):
                            tiles += [ohb_lo[:], ohb_hi[:]]
                        for t_ in tiles:
                            rr = wpool.tile([P, 1], f32, tag="rd", name="rd")
                            nc.vector.reduce_sum(out=rr[:], in_=t_,
                                                 axis=X_)
                            nc.vector.tensor_tensor(
                                out=Tdst[:, g:g + 1], in0=rr[:],
                                in1=rr[:], op=mybir.AluOpType.add)
                        if g + 2 < NGRP:
                            issue_lo(g + 2)
                        if g + 1 < NGRP:
                            issue_hi(g + 1)
                        continue
                    for wi in range(GW):
                        w = g * GW + wi
                        nl = capl[w]
                        nh = caph[w]
                        ntot = nl + nh
                        lo_off = ckl[w] - kbl[g]   # chunk offset in group
                        hi_off = ckh[w] - kbh[g]

                        def chunk_aps(k, nl=nl, lo_off=lo_off,
                                      hi_off=hi_off, mlo=mlo, mhi=mhi,
                                      ohb_lo=ohb_lo, ohb_hi=ohb_hi):
                            if k < nl:
                                c = lo_off + k
                                return (mlo[:, c * P:(c + 1) * P],
                                        ohb_lo[:, c * P:(c + 1) * P])
                            c = hi_off + (k - nl)
                            return (mhi[:, c * P:(c + 1) * P],
                                    ohb_hi[:, c * P:(c + 1) * P])

                        # self-loop rows of this window, read back from the
                        # local G shard (contiguous 32KB, plain DMA)
                        gsl = wpool.tile([P, P], bf16, tag="gsl", name="gsl")
                        if w < SPLIT_W:
                            nc.sync.dma_start(
                                out=gsl[:],
                                in_=glocA[l][w * P:(w + 1) * P, :])
                        else:
                            wb = w - SPLIT_W
                            nc.sync.dma_start(
                                out=gsl[:],
                                in_=glocB[l][wb * P:(wb + 1) * P, :])

                        # batch transposes 4-to-a-PSUM-bank ahead of the
                        # accumulate chain; one wide DVE copy per bank moves
                        # them to SBUF so mm2 never stalls on the round trip
                        TB = 4
                        nbk = (ntot + TB - 1) // TB
                        msgs = []
                        if not opts.get("fixed_msg"):
                            for b in range(nbk):
                                kk = min(TB, ntot - b * TB)
                                psT = tpool.tile([P, TB * P], bf16, tag="tr",
                                                 name="psT")
                                for j in range(kk):
                                    mT_ap, _ = chunk_aps(b * TB + j)
                                    nc.tensor.transpose(
                                        psT[:, j * P:(j + 1) * P], mT_ap,
                                        ident_sb[:])
                                msg = mspool.tile([P, TB * P], bf16,
                                                  tag="msg", name="msg")
                                nc.vector.tensor_scalar(
                                    out=msg[:, 0:kk * P],
                                    in0=psT[:, 0:kk * P],
                                    scalar1=1.0, scalar2=None,
                                    op0=mybir.AluOpType.mult)
                                msgs.append(msg)
                        ps = ppool.tile([P, P], f32, tag="acc", name="pss")
                        for k in range(ntot):
                            _, oh_ap = chunk_aps(k)
                            if opts.get("fixed_msg"):
                                m_ap = ohb_lo[:, 0:P]
                            else:
                                m_ap = msgs[k // TB][:, (k % TB) * P:
                                                     (k % TB + 1) * P]
                            nc.tensor.matmul(ps[:], lhsT=m_ap,
                                             rhs=oh_ap,
                                             start=(k == 0), stop=False)
                        # self-loop: ps[f, r] += G_l[w*128+r, f]
                        nc.tensor.matmul(ps[:], lhsT=gsl[:], rhs=ident_sb[:],
                                         start=False, stop=True)
                        leaky_into(Tdst[:, w * P:(w + 1) * P], ps)
                    if g + 2 < NGRP:
                        issue_lo(g + 2)
                    if g + 1 < NGRP:
                        issue_hi(g + 1)

            def head(Tsrc):
                X = mybir.AxisListType.X
                Exp = mybir.ActivationFunctionType.Exp
                for w in range(NW):
                    y1t = []
                    for h in range(2):
                        ps1 = ppool.tile([P, P], f32, tag="acc", name="ps1")
                        nc.tensor.matmul(ps1[:],
                                         lhsT=wfc1_sb[:, h * P:(h + 1) * P],
                                         rhs=Tsrc[:, w * P:(w + 1) * P],
                                         start=True, stop=True)
                        yt = wpool.tile([P, P], bf16, tag=f"y1_{h}",
                                        name="yt")
                        leaky_into(yt[:], ps1)
                        y1t.append(yt)
                    ps2 = p2pool.tile([P, 2], f32, tag="y2", name="ps2")
                    nc.tensor.matmul(ps2[:], lhsT=y1t[0][:],
                                     rhs=wfc2_sb[:, 0:2],
                                     start=True, stop=False)
                    nc.tensor.matmul(ps2[:], lhsT=y1t[1][:],
                                     rhs=wfc2_sb[:, 2:4],
                                     start=False, stop=True)
                    y2 = wpool.tile([P, 2], f32, tag="y2s", name="y2")
                    leaky_into(y2[:], ps2)
                    z = wpool.tile([P, 2], f32, tag="z", name="z")
                    nc.scalar.activation(z[:], y2[:], Copy, bias=0.0,
                                         scale=dinv[:, w:w + 1])
                    negm = wpool.tile([P, 1], f32, tag="m", name="negm")
                    nc.vector.reduce_max(out=negm[:], in_=z[:], axis=X,
                                         negate=True)
                    e = wpool.tile([P, 2], f32, tag="e", name="e")
                    nc.scalar.activation(e[:], z[:], Exp,
                                         bias=negm[:, 0:1], scale=1.0)
                    s = wpool.tile([P, 1], f32, tag="s", name="s")
                    nc.vector.reduce_sum(out=s[:], in_=e[:], axis=X)
                    rs = wpool.tile([P, 1], f32, tag="rs", name="rs")
                    nc.vector.reciprocal(rs[:], s[:])
                    o = wpool.tile([P, 2], f32, tag="o", name="o")
                    nc.vector.tensor_scalar(out=o[:], in0=e[:],
                                            scalar1=rs[:, 0:1], scalar2=None,
                                            op0=mybir.AluOpType.mult)
                    nc.sync.dma_start(out=out_d[w * P:(w + 1) * P, :],
                                      in_=o[:])

            sc = scatter_tr
            for _rep in range(reps):
                if opts.get("skip_scatter"):
                    g_production(0, T_a)
                    g_production(1, T_a)
                    g_production(2, T_a)
                    head(T_a)
                else:
                    g_production(0, T_a)
                    sc(0, T_b)
                    g_production(1, T_b)
                    sc(1, T_a)
                    g_production(2, T_a)
                    sc(2, T_b)
                    head(T_b)

    nc.compile()
    return nc


# --------------------------------------------------------------------------
# Entry point
# --------------------------------------------------------------------------
LAST_RESULT = None
LAST_NC = None
LAST_IN_MAPS = None
LAST_META = None


def kernel(x, edge_index, Wg0, Wg1, Wg2, Wfc1, Wfc2):
    from concourse.bass_utils import run_bass_kernel_spmd

    global LAST_RESULT, LAST_NC, LAST_IN_MAPS, LAST_META
    x = np.asarray(x)
    edge_index = np.asarray(edge_index)
    per_core, meta = _preprocess(x, edge_index)
    in_maps = _build_core_inputs(
        x, (np.asarray(Wg0), np.asarray(Wg1), np.asarray(Wg2),
            np.asarray(Wfc1), np.asarray(Wfc2)), per_core, meta)
    nc = _build_bass(meta)
    LAST_NC, LAST_IN_MAPS, LAST_META = nc, in_maps, meta
    res = run_bass_kernel_spmd(nc, in_maps, core_ids=list(range(N_CORES)))
    LAST_RESULT = res
    NL = meta["NL"]
    out = np.concatenate([res.results[c]["out"][:NL] for c in range(N_CORES)],
                         axis=0)
    return out.astype(np.float32)



# revision 13
# speedup vs baseline: 4.8247x; 4.8247x over previous
"""GCN message-passing kernel for 8 Trainium2 NeuronCores.

Model (PyG GCNConv x3 + MLP head + softmax):
    A01 = adjacency + self loops (unit weights), deg = in-degree over A01
    conv(H, W) = D^-1/2 A01 D^-1/2 (H @ W)
    h = x; h = leaky(conv(h, Wg_l)) x3
    y = softmax(leaky(leaky(h @ Wfc1) @ Wfc2))

Key algebraic rewrite: leaky_relu is positively homogeneous, so the
D^-1/2 factors can be pulled out of every leaky() and folded into the
per-layer "message table" G_l:
    G_1 = D^-1/2 (x @ Wg0)
    Ht_{l+1} = leaky(A01 @ G_l)            (pure 0/1 segment-sum!)
    G_{l+1} = D^-1 (Ht_{l+1} @ Wg_l)
    final: z = D^-1/2 leaky(leaky(Ht_4 @ Wfc1) @ Wfc2), out = softmax(z)

Sharding: destination nodes are split into 8 contiguous blocks of 6250
(padded to 6272 = 49 windows of 128). Each layer: every core computes its
G shard (matmul + per-row scale), an AllGather builds the full G table in
DRAM, then each core gathers source rows for its edges with
dma_gather(transpose=True) (int16 indices sorted ascending per bucket for
DRAM locality; table split in two <32768-row halves; ragged per-window
chunk caps = max over cores). The feature-major gather output is
PE-transposed back to edge-major 4 chunks per PSUM bank, moved to SBUF by
one wide DVE copy, and accumulated  msg^T @ onehot  into a PSUM window on
the TensorEngine (one-hot built on DVE via is_equal vs an iota row).
Self-loop messages skip the gather entirely: one identity-rhs matmul adds
the window's own G rows (read back contiguously from the local shard).
The flush produces the next layer's activations already transposed
(feat x rows), which is exactly the lhsT layout the next matmul needs.
"""

import numpy as np

P = 128
N_CORES = 8


def _gw(NW):
    """Windows per gather group (last group may be ragged).  Large groups
    amortize the ~20us per-gather-instruction floor; the SWDGE ring caps a
    single gather at ~8144 idxs."""
    return 7 if NW % 7 == 0 else 1


# --------------------------------------------------------------------------
# Host-side preprocessing: shard edges by destination, pad to fixed chunk
# counts (SPMD requires an identical instruction stream on all cores).
# --------------------------------------------------------------------------
def _preprocess(x, edge_index):
    N, D = x.shape
    assert D == P
    NL = N // N_CORES                      # real nodes per core
    NW = (NL + P - 1) // P                 # windows per core
    NLP = NW * P                           # padded nodes per core
    NGP = N_CORES * NLP                    # padded global nodes
    # A/B source-table split at the per-core window midpoint: table A holds
    # every core's windows [0, SPLIT_W), table B the rest.  Both stay under
    # the 32768-row int16 gather limit, and the A-half AllGather can fire
    # as soon as each core finishes producing its first SPLIT_W windows,
    # overlapping the B-half production and the B AllGather with the
    # A-half gathers.
    SPLIT_W = (NW + 1) // 2
    SPLIT = SPLIT_W * P

    src = np.asarray(edge_index[0], dtype=np.int64)
    dst = np.asarray(edge_index[1], dtype=np.int64)

    # in-degree INCLUDING the self loop (GCNConv adds A+I); the self-loop
    # message itself is applied on-device via an identity matmul over the
    # local G rows, so loop edges are NOT bucketed.
    deg = (np.bincount(dst, minlength=N) + 1).astype(np.float32)

    sowner = src // NL
    lid_s = src - sowner * NL              # source local id on its core

    owner = dst // NL                      # destination owner core
    lid = dst - owner * NL                 # local dest id on that core
    w = lid // P                           # window
    dr = (lid % P).astype(np.float32)      # one-hot row within window
    half = (lid_s >= SPLIT).astype(np.int64)
    srel = np.where(half == 1,
                    sowner * (NLP - SPLIT) + (lid_s - SPLIT),
                    sowner * SPLIT + lid_s)  # row within table A/B
    assert N_CORES * SPLIT <= 32768 and N_CORES * (NLP - SPLIT) <= 32768

    # bucket key: (core, half, window); sort by srel within each bucket so
    # the gather walks ascending rows (DRAM page locality)
    key = ((owner * 2 + half) * NW + w)
    nbuckets = N_CORES * 2 * NW
    order = np.lexsort((srel, key))
    key_s = key[order]
    srel_s = srel[order]
    dr_s = dr[order]

    counts = np.bincount(key_s, minlength=nbuckets).reshape(N_CORES, 2, NW)
    # ragged per-window chunk caps (max over cores, shared SPMD stream)
    capl = np.ceil(counts[:, 0, :].max(axis=0) / P).astype(np.int64)  # [NW]
    caph = np.ceil(counts[:, 1, :].max(axis=0) / P).astype(np.int64)
    capl = np.maximum(capl, 1)   # keep streams non-empty (small graphs)
    caph = np.maximum(caph, 1)
    CAPW = np.stack([capl, caph], axis=0) * P          # [2, NW] slots
    TL = int(CAPW[0].sum())                            # lo slots per core
    TH = int(CAPW[1].sum())

    # per-(core, half, window) slot base inside that core's lo/hi stream
    wbase = np.zeros((2, NW), dtype=np.int64)
    wbase[0, 1:] = np.cumsum(CAPW[0])[:-1]
    wbase[1, 1:] = np.cumsum(CAPW[1])[:-1]

    start = np.zeros(nbuckets, dtype=np.int64)
    start[1:] = np.cumsum(counts.reshape(-1))[:-1]
    within = np.arange(len(key_s)) - start[key_s]
    h_s = (key_s // NW) % 2
    w_s = key_s % NW
    c_s = key_s // (2 * NW)
    dest = c_s * (TL + TH) + np.where(h_s == 0, 0, TL) \
        + wbase[h_s, w_s] + within

    total_cap = N_CORES * (TL + TH)
    idx_flat = np.zeros(total_cap, dtype=np.int16)
    dr_flat = np.full(total_cap, 200.0, dtype=np.float32)
    idx_flat[dest] = srel_s.astype(np.int16)
    dr_flat[dest] = dr_s

    per_core = []
    for c in range(N_CORES):
        seg_i = idx_flat[c * (TL + TH):(c + 1) * (TL + TH)]
        seg_d = dr_flat[c * (TL + TH):(c + 1) * (TL + TH)]
        ilo, ihi = seg_i[:TL], seg_i[TL:]
        dlo, dhi = seg_d[:TL], seg_d[TL:]
        per_core.append((ilo, ihi, dlo, dhi))

    meta = dict(N=N, NL=NL, NW=NW, NLP=NLP, NGP=NGP, SPLIT_W=SPLIT_W,
                capl=[int(v) for v in capl], caph=[int(v) for v in caph],
                TL=TL, TH=TH, deg=deg)
    return per_core, meta


def _wrap_idx_ragged(flat, gbounds):
    """flat: [T] int16 slot stream (window-major, ragged caps).
    gbounds: slot offsets of each gather-group boundary (len ngrp+1).
    Returns [128, T/16] int16 in dma_gather's wrapped layout: within each
    group block, logical index j lives at [j % 16, j // 16], replicated
    8x across the 128 partitions."""
    cols = []
    for g in range(len(gbounds) - 1):
        block = flat[gbounds[g]:gbounds[g + 1]]
        m = block.reshape(-1, 16).T
        cols.append(np.tile(m, (8, 1)))
    return np.ascontiguousarray(np.concatenate(cols, axis=1))


def _group_bounds(cap, GW):
    """Slot/chunk offsets per gather group given per-window caps."""
    NW = len(cap)
    ngrp = (NW + GW - 1) // GW
    sb = [0]
    kb = [0]
    for g in range(ngrp):
        sb.append(sb[-1] + sum(cap[g * GW:(g + 1) * GW]) * P)
        kb.append(kb[-1] + sum(cap[g * GW:(g + 1) * GW]))
    return sb, kb


def _build_core_inputs(x, Ws, per_core, meta):
    """Build the per-core device input dict."""
    N, NL, NW, NLP = meta["N"], meta["NL"], meta["NW"], meta["NLP"]
    capl, caph = meta["capl"], meta["caph"]
    deg = meta["deg"]
    GW = _gw(NW)
    Wg0, Wg1, Wg2, Wfc1, Wfc2 = Ws

    iota = np.tile(np.arange(P, dtype=np.float32), (P, 1))
    ident = np.eye(P, dtype=np.float32)
    # Wfc2 [256, 2] -> [128, 4]: cols 0:2 first half of u, 2:4 second half
    Wfc2p = np.concatenate([Wfc2[:P, :], Wfc2[P:, :]], axis=1)
    Wfc2p = np.ascontiguousarray(Wfc2p, dtype=np.float32)

    sbl, _ = _group_bounds(capl, GW)
    sbh, _ = _group_bounds(caph, GW)

    def dstrel(dflat, cap):
        # [128, sum(cap)]: col = chunk (window-major), row p = edge slot
        cols = []
        o = 0
        for w_ in range(NW):
            n = cap[w_] * P
            cols.append(dflat[o:o + n].reshape(cap[w_], P).T)
            o += n
        return np.ascontiguousarray(np.concatenate(cols, axis=1))

    in_maps = []
    for c in range(N_CORES):
        ilo, ihi, dlo, dhi = per_core[c]
        xs = np.zeros((NLP, P), dtype=np.float32)
        xs[:NL] = x[c * NL:(c + 1) * NL]
        x_t = np.ascontiguousarray(xs.T)                   # [128, NLP]

        degp = np.ones(NLP, dtype=np.float32)
        degp[:NL] = deg[c * NL:(c + 1) * NL]
        deg_t = np.ascontiguousarray(degp.reshape(NW, P).T)  # [128, NW]

        in_maps.append({
            "x_t": x_t,
            "deg_t": deg_t,
            "idx_lo": _wrap_idx_ragged(ilo, sbl),
            "dstrel_lo": dstrel(dlo, capl),
            "idx_hi": _wrap_idx_ragged(ihi, sbh),
            "dstrel_hi": dstrel(dhi, caph),
            "iota": iota,
            "ident": ident,
            "Wg0": np.ascontiguousarray(Wg0, dtype=np.float32),
            "Wg1": np.ascontiguousarray(Wg1, dtype=np.float32),
            "Wg2": np.ascontiguousarray(Wg2, dtype=np.float32),
            "Wfc1": np.ascontiguousarray(Wfc1, dtype=np.float32),
            "Wfc2p": Wfc2p,
        })
    return in_maps


# --------------------------------------------------------------------------
# Device program
# --------------------------------------------------------------------------
def _build_bass(meta, mock_cc=False, opts=None, reps=1):
    opts = opts or {}
    from concourse import bass, bacc, mybir
    import concourse.tile as tile

    NW, NLP, NGP = meta["NW"], meta["NLP"], meta["NGP"]
    SPLIT_W = meta["SPLIT_W"]
    SPLIT = SPLIT_W * P
    NBW = NW - SPLIT_W                     # B-half windows (may be 0)
    NA = N_CORES * SPLIT                   # table A rows
    NB = N_CORES * (NLP - SPLIT)           # table B rows
    capl, caph = meta["capl"], meta["caph"]
    TL, TH = meta["TL"], meta["TH"]
    GW = _gw(NW)
    NGRP = (NW + GW - 1) // GW
    sbl, kbl = _group_bounds(capl, GW)     # slot / chunk offsets per group
    sbh, kbh = _group_bounds(caph, GW)
    ckl = np.concatenate([[0], np.cumsum(capl)]).astype(int)  # per window
    ckh = np.concatenate([[0], np.cumsum(caph)]).astype(int)
    GLMAXL = max(sbl[g + 1] - sbl[g] for g in range(NGRP))
    GLMAXH = max(sbh[g + 1] - sbh[g] for g in range(NGRP))
    KMAXL = max(kbl[g + 1] - kbl[g] for g in range(NGRP))
    KMAXH = max(kbh[g + 1] - kbh[g] for g in range(NGRP))
    f32 = mybir.dt.float32
    bf16 = mybir.dt.bfloat16
    i16 = mybir.dt.int16
    ALL = [list(range(N_CORES))]

    nc = bacc.Bacc("TRN2", target_bir_lowering=False, debug=False,
                   num_devices=N_CORES)

    x_t_d = nc.dram_tensor("x_t", [P, NLP], f32, kind="ExternalInput")
    deg_d = nc.dram_tensor("deg_t", [P, NW], f32, kind="ExternalInput")
    ilo_d = nc.dram_tensor("idx_lo", [P, TL // 16], i16, kind="ExternalInput")
    drl_d = nc.dram_tensor("dstrel_lo", [P, TL // P], f32,
                           kind="ExternalInput")
    ihi_d = nc.dram_tensor("idx_hi", [P, TH // 16], i16, kind="ExternalInput")
    drh_d = nc.dram_tensor("dstrel_hi", [P, TH // P], f32,
                           kind="ExternalInput")
    iota_d = nc.dram_tensor("iota", [P, P], f32, kind="ExternalInput")
    ident_d = nc.dram_tensor("ident", [P, P], f32, kind="ExternalInput")
    wg_d = [nc.dram_tensor(f"Wg{i}", [P, P], f32, kind="ExternalInput")
            for i in range(3)]
    wfc1_d = nc.dram_tensor("Wfc1", [P, 256], f32, kind="ExternalInput")
    wfc2_d = nc.dram_tensor("Wfc2p", [P, 4], f32, kind="ExternalInput")
    out_d = nc.dram_tensor("out", [NLP, 2], f32, kind="ExternalOutput")

    with tile.TileContext(nc) as tc:
        with (
            tc.tile_pool(name="const", bufs=1) as cpool,
            tc.tile_pool(name="msg", bufs=3) as mpool,
            tc.tile_pool(name="oh", bufs=3) as ohpool,
            tc.tile_pool(name="work", bufs=3) as wpool,
            tc.tile_pool(name="acc", bufs=3, space="PSUM") as ppool,
            tc.tile_pool(name="accy", bufs=1, space="PSUM") as p2pool,
            tc.tile_pool(name="tpsum", bufs=4, space="PSUM") as tpool,
            tc.tile_pool(name="msgs", bufs=8) as mspool,
            tc.tile_pool(name="dram", bufs=1, space="DRAM") as dpool,
        ):
            # ---- constants / casts ----
            T_a = cpool.tile([P, NLP], bf16, name="T_a")
            nc.gpsimd.dma_start(out=T_a[:], in_=x_t_d[:])   # f32->bf16 cast
            T_b = cpool.tile([P, NLP], bf16, name="T_b")

            iota_sb = cpool.tile([P, P], bf16, name="iota_sb")
            nc.gpsimd.dma_start(out=iota_sb[:], in_=iota_d[:])
            ident_sb = cpool.tile([P, P], bf16, name="ident_sb")
            nc.gpsimd.dma_start(out=ident_sb[:], in_=ident_d[:])
            wg_sb = []
            for i in range(3):
                t = cpool.tile([P, P], bf16, name=f"wg_sb{i}")
                nc.gpsimd.dma_start(out=t[:], in_=wg_d[i][:])
                wg_sb.append(t)
            wfc1_sb = cpool.tile([P, 256], bf16, name="wfc1_sb")
            nc.gpsimd.dma_start(out=wfc1_sb[:], in_=wfc1_d[:])
            wfc2_sb = cpool.tile([P, 4], bf16, name="wfc2_sb")
            nc.gpsimd.dma_start(out=wfc2_sb[:], in_=wfc2_d[:])
            drl_sb = cpool.tile([P, TL // P], bf16, name="drl_sb")
            nc.gpsimd.dma_start(out=drl_sb[:], in_=drl_d[:])  # f32->bf16
            ilo_sb = cpool.tile([P, TL // 16], i16, name="ilo_sb")
            nc.sync.dma_start(out=ilo_sb[:], in_=ilo_d[:])
            drh_sb = cpool.tile([P, TH // P], bf16, name="drh_sb")
            nc.gpsimd.dma_start(out=drh_sb[:], in_=drh_d[:])
            ihi_sb = cpool.tile([P, TH // 16], i16, name="ihi_sb")
            nc.sync.dma_start(out=ihi_sb[:], in_=ihi_d[:])

            deg_sb = cpool.tile([P, NW], f32, name="deg_sb")
            nc.sync.dma_start(out=deg_sb[:], in_=deg_d[:])
            invdeg = cpool.tile([P, NW], f32, name="invdeg")
            nc.vector.reciprocal(invdeg[:], deg_sb[:])
            dinv = cpool.tile([P, NW], f32, name="dinv")
            nc.scalar.sqrt(dinv[:], invdeg[:])

            # NB: collective outputs in Local addr space — Shared
            # scratchpad DMA reads measured ~3x slower on the gather path.
            # Separate A/B local shards + tables give the Tile scheduler
            # precise deps: AG_A fires after the first SPLIT_W windows of
            # production and overlaps the rest; AG_B overlaps the A-half
            # gathers of the scatter phase.
            gfA = [dpool.tile([NA, P], bf16, name=f"gfA{i}") for i in range(3)]
            gfB = ([dpool.tile([NB, P], bf16, name=f"gfB{i}") for i in range(3)]
                   if NBW else None)
            glocA = [dpool.tile([SPLIT, P], bf16, name=f"glocA{i}")
                     for i in range(3)]
            glocB = ([dpool.tile([NLP - SPLIT, P], bf16, name=f"glocB{i}")
                      for i in range(3)] if NBW else None)

            Copy = mybir.ActivationFunctionType.Copy

            # retained node-major G tables (self-loop rows; double-buffered
            # across layers) — replaces per-window DRAM readbacks
            gwkeep = [cpool.tile([P, NLP], bf16, name=f"gwkeep{i}")
                      for i in range(2)]

            def prod_window(l, Tsrc, w):
                """Produce G_l window w (node-major) from activations Tsrc:
                matmul + per-row scale into the retained SBUF table, DMA the
                shard row-block out, and fire the A/B AllGathers at the split
                points.  Called fused from the previous layer's scatter so
                the collectives overlap the tail of that layer's gathers."""
                scale = dinv if l == 0 else invdeg
                ps = ppool.tile([P, P], f32, tag="acc", name="psg")
                nc.tensor.matmul(ps[:], lhsT=Tsrc[:, w * P:(w + 1) * P],
                                 rhs=wg_sb[l][:], start=True, stop=True)
                gk = gwkeep[l % 2]
                nc.scalar.activation(gk[:, w * P:(w + 1) * P], ps[:], Copy,
                                     bias=0.0, scale=scale[:, w:w + 1])
                if w < SPLIT_W:
                    nc.sync.dma_start(out=glocA[l][w * P:(w + 1) * P, :],
                                      in_=gk[:, w * P:(w + 1) * P])
                else:
                    wb = w - SPLIT_W
                    nc.sync.dma_start(out=glocB[l][wb * P:(wb + 1) * P, :],
                                      in_=gk[:, w * P:(w + 1) * P])
                if w == SPLIT_W - 1:
                    if mock_cc:
                        for c in range(N_CORES):
                            nc.sync.dma_start(
                                out=gfA[l][c * SPLIT:(c + 1) * SPLIT, :],
                                in_=glocA[l][:])
                    else:
                        nc.gpsimd.collective_compute(
                            "AllGather", mybir.AluOpType.bypass,
                            replica_groups=ALL,
                            ins=[glocA[l][:]], outs=[gfA[l][:]])
                if w == NW - 1 and NBW:
                    if mock_cc:
                        nsh = NLP - SPLIT
                        for c in range(N_CORES):
                            nc.sync.dma_start(
                                out=gfB[l][c * nsh:(c + 1) * nsh, :],
                                in_=glocB[l][:])
                    else:
                        nc.gpsimd.collective_compute(
                            "AllGather", mybir.AluOpType.bypass,
                            replica_groups=ALL,
                            ins=[glocB[l][:]], outs=[gfB[l][:]])

            def leaky_into(dst_ap, ps):
                t = wpool.tile([P, dst_ap.shape[-1]], f32, tag="lk", name="lkt")
                nc.scalar.activation(t[:], ps[:], Copy, bias=0.0, scale=0.01)
                nc.vector.tensor_tensor(out=dst_ap, in0=ps[:], in1=t[:],
                                        op=mybir.AluOpType.max)


            def scatter_tr(l, Tdst, last=False):
                """Transpose-mode gather (dma_gather(transpose=True)) returns
                messages feature-major [128f, n_idxs]; each 128-edge chunk is
                PE-transposed back to edge-major via the identity trick,
                staged through PSUM(bf16) -> SBUF (Act copy), then accumulated
                with the one-hot matmul.  Chunk counts are ragged per window;
                the self-loop message is injected exactly via an identity-rhs
                matmul over the retained SBUF G rows (no gather slots, no
                DRAM readback).  When not `last`, the next layer's production
                for each window is emitted right after its leaky, so the
                AllGathers fire while this layer's gathers still stream."""
                glo_ap = gfA[l][:]
                # degenerate graphs (single window) still issue B gathers
                # with idx 0 and all-dead one-hot columns
                ghi_ap = gfB[l][:] if NBW else gfA[l][0:P, :]
                gk = gwkeep[l % 2]

                # The Pool engine runs gathers in emission order, and every
                # B-table gather waits on AG_B.  Emit lo a group ahead of hi
                # so A-side gathers keep streaming while AG_B is in flight.
                mtiles = {}

                def issue_lo(g):
                    if g >= NGRP:
                        return
                    GLLg = sbl[g + 1] - sbl[g]
                    t = mpool.tile([P, GLMAXL], bf16, tag="mlo", name="mlo")
                    if not opts.get("skip_gather"):
                        nc.gpsimd.dma_gather(
                            out_ap=t[:, 0:GLLg].rearrange("p (c e) -> p c e",
                                                          c=1),
                            in_ap=glo_ap,
                            idxs_ap=ilo_sb[:, sbl[g] // 16:sbl[g + 1] // 16],
                            num_idxs=GLLg, num_idxs_reg=GLLg, elem_size=P,
                            transpose=True, single_packet=False)
                    mtiles[("lo", g)] = t

                def issue_hi(g):
                    if g >= NGRP:
                        return
                    GLHg = sbh[g + 1] - sbh[g]
                    t = mpool.tile([P, GLMAXH], bf16, tag="mhi", name="mhi")
                    if not opts.get("skip_gather"):
                        nc.gpsimd.dma_gather(
                            out_ap=t[:, 0:GLHg].rearrange("p (c e) -> p c e",
                                                          c=1),
                            in_ap=ghi_ap,
                            idxs_ap=ihi_sb[:, sbh[g] // 16:sbh[g + 1] // 16],
                            num_idxs=GLHg, num_idxs_reg=GLHg, elem_size=P,
                            transpose=True, single_packet=False)
                    mtiles[("hi", g)] = t

                issue_lo(0)
                issue_lo(1)
                issue_lo(2)
                issue_hi(0)
                issue_hi(1)
                OHL = max(capl) * P
                OHH = max(caph) * P
                for g in range(NGRP):
                    mlo = mtiles.pop(("lo", g))
                    mhi = mtiles.pop(("hi", g))
                    if opts.get("skip_matmul"):
                        # timing-only variant: consume gathers/onehots with
                        # cheap DVE reduces so nothing dead-codes
                        X_ = mybir.AxisListType.X
                        tiles = []
                        if not opts.get("skip_gather"):
                            tiles += [mlo[:, 0:P], mhi[:, 0:P]]
                        if not opts.get("skip_onehot"):
                            for w in range(g * GW, min((g + 1) * GW, NW)):
                                ohw = ohpool.tile([P, OHL], bf16, tag="ohlo",
                                                  name="ohw_lo")
                                nc.vector.tensor_tensor(
                                    out=ohw[:, 0:capl[w] * P].rearrange(
                                        "p (k r) -> p k r", r=P),
                                    in0=iota_sb[:].unsqueeze(1)
                                        .to_broadcast([P, capl[w], P]),
                                    in1=drl_sb[:, ckl[w]:ckl[w + 1]]
                                        .unsqueeze(2)
                                        .to_broadcast([P, capl[w], P]),
                                    op=mybir.AluOpType.is_equal)
                                tiles.append(ohw[:, 0:capl[w] * P])
                        for t_ in tiles:
                            rr = wpool.tile([P, 1], f32, tag="rd", name="rd")
                            nc.vector.reduce_sum(out=rr[:], in_=t_,
                                                 axis=X_)
                            nc.vector.tensor_tensor(
                                out=Tdst[:, g:g + 1], in0=rr[:],
                                in1=rr[:], op=mybir.AluOpType.add)
                        issue_lo(g + 3)
                        issue_hi(g + 2)
                        continue
                    for w in range(g * GW, min((g + 1) * GW, NW)):
                        nl = capl[w]
                        nh = caph[w]
                        ntot = nl + nh
                        lo_off = ckl[w] - kbl[g]   # chunk offset in group
                        hi_off = ckh[w] - kbh[g]

                        # per-window one-hots (small tiles keep SBUF free
                        # for deep gather buffering)
                        ohw_lo = ohpool.tile([P, OHL], bf16, tag="ohlo",
                                             name="ohw_lo")
                        ohw_hi = ohpool.tile([P, OHH], bf16, tag="ohhi",
                                             name="ohw_hi")
                        nc.vector.tensor_tensor(
                            out=ohw_lo[:, 0:nl * P].rearrange(
                                "p (k r) -> p k r", r=P),
                            in0=iota_sb[:].unsqueeze(1)
                                .to_broadcast([P, nl, P]),
                            in1=drl_sb[:, ckl[w]:ckl[w + 1]].unsqueeze(2)
                                .to_broadcast([P, nl, P]),
                            op=mybir.AluOpType.is_equal)
                        nc.vector.tensor_tensor(
                            out=ohw_hi[:, 0:nh * P].rearrange(
                                "p (k r) -> p k r", r=P),
                            in0=iota_sb[:].unsqueeze(1)
                                .to_broadcast([P, nh, P]),
                            in1=drh_sb[:, ckh[w]:ckh[w + 1]].unsqueeze(2)
                                .to_broadcast([P, nh, P]),
                            op=mybir.AluOpType.is_equal)

                        def chunk_aps(k, nl=nl, lo_off=lo_off,
                                      hi_off=hi_off, mlo=mlo, mhi=mhi,
                                      ohw_lo=ohw_lo, ohw_hi=ohw_hi):
                            if k < nl:
                                c = lo_off + k
                                return (mlo[:, c * P:(c + 1) * P],
                                        ohw_lo[:, k * P:(k + 1) * P])
                            c = hi_off + (k - nl)
                            return (mhi[:, c * P:(c + 1) * P],
                                    ohw_hi[:, (k - nl) * P:(k - nl + 1) * P])

                        # batch transposes 4-to-a-PSUM-bank ahead of the
                        # accumulate chain; one wide Act copy per bank moves
                        # them to SBUF so mm2 never stalls on the round trip
                        TB = 4
                        nbk = (ntot + TB - 1) // TB
                        msgs = []
                        if not opts.get("fixed_msg"):
                            for b in range(nbk):
                                kk = min(TB, ntot - b * TB)
                                psT = tpool.tile([P, TB * P], bf16, tag="tr",
                                                 name="psT")
                                for j in range(kk):
                                    mT_ap, _ = chunk_aps(b * TB + j)
                                    nc.tensor.transpose(
                                        psT[:, j * P:(j + 1) * P], mT_ap,
                                        ident_sb[:])
                                msg = mspool.tile([P, TB * P], bf16,
                                                  tag="msg", name="msg")
                                nc.scalar.activation(
                                    msg[:, 0:kk * P], psT[:, 0:kk * P],
                                    Copy, bias=0.0, scale=1.0)
                                msgs.append(msg)
                        ps = ppool.tile([P, P], f32, tag="acc", name="pss")
                        for k in range(ntot):
                            _, oh_ap = chunk_aps(k)
                            if opts.get("fixed_msg"):
                                m_ap = ohw_lo[:, 0:P]
                            else:
                                m_ap = msgs[k // TB][:, (k % TB) * P:
                                                     (k % TB + 1) * P]
                            nc.tensor.matmul(ps[:], lhsT=m_ap,
                                             rhs=oh_ap,
                                             start=(k == 0), stop=False)
                        # self-loop: ps[f, r] += G_l[w*128+r, f] from the
                        # retained SBUF table (no DRAM readback)
                        nc.tensor.matmul(ps[:], lhsT=gk[:, w * P:(w + 1) * P],
                                         rhs=ident_sb[:],
                                         start=False, stop=True)
                        leaky_into(Tdst[:, w * P:(w + 1) * P], ps)
                        if not last:
                            prod_window(l + 1, Tdst, w)
                    issue_lo(g + 3)
                    issue_hi(g + 2)

            def head(Tsrc):
                X = mybir.AxisListType.X
                Exp = mybir.ActivationFunctionType.Exp
                for w in range(NW):
                    y1t = []
                    for h in range(2):
                        ps1 = ppool.tile([P, P], f32, tag="acc", name="ps1")
                        nc.tensor.matmul(ps1[:],
                                         lhsT=wfc1_sb[:, h * P:(h + 1) * P],
                                         rhs=Tsrc[:, w * P:(w + 1) * P],
                                         start=True, stop=True)
                        yt = wpool.tile([P, P], bf16, tag=f"y1_{h}",
                                        name="yt")
                        leaky_into(yt[:], ps1)
                        y1t.append(yt)
                    ps2 = p2pool.tile([P, 2], f32, tag="y2", name="ps2")
                    nc.tensor.matmul(ps2[:], lhsT=y1t[0][:],
                                     rhs=wfc2_sb[:, 0:2],
                                     start=True, stop=False)
                    nc.tensor.matmul(ps2[:], lhsT=y1t[1][:],
                                     rhs=wfc2_sb[:, 2:4],
                                     start=False, stop=True)
                    y2 = wpool.tile([P, 2], f32, tag="y2s", name="y2")
                    leaky_into(y2[:], ps2)
                    z = wpool.tile([P, 2], f32, tag="z", name="z")
                    nc.scalar.activation(z[:], y2[:], Copy, bias=0.0,
                                         scale=dinv[:, w:w + 1])
                    negm = wpool.tile([P, 1], f32, tag="m", name="negm")
                    nc.vector.reduce_max(out=negm[:], in_=z[:], axis=X,
                                         negate=True)
                    e = wpool.tile([P, 2], f32, tag="e", name="e")
                    nc.scalar.activation(e[:], z[:], Exp,
                                         bias=negm[:, 0:1], scale=1.0)
                    s = wpool.tile([P, 1], f32, tag="s", name="s")
                    nc.vector.reduce_sum(out=s[:], in_=e[:], axis=X)
                    rs = wpool.tile([P, 1], f32, tag="rs", name="rs")
                    nc.vector.reciprocal(rs[:], s[:])
                    o = wpool.tile([P, 2], f32, tag="o", name="o")
                    nc.vector.tensor_scalar(out=o[:], in0=e[:],
                                            scalar1=rs[:, 0:1], scalar2=None,
                                            op0=mybir.AluOpType.mult)
                    nc.sync.dma_start(out=out_d[w * P:(w + 1) * P, :],
                                      in_=o[:])

            sc = scatter_tr
            for _rep in range(reps):
                if opts.get("skip_scatter"):
                    for l in range(3):
                        for w in range(NW):
                            prod_window(l, T_a, w)
                    head(T_a)
                else:
                    for w in range(NW):
                        prod_window(0, T_a, w)
                    sc(0, T_b)             # fused: also produces G_1
                    sc(1, T_a)             # fused: also produces G_2
                    sc(2, T_b, last=True)
                    head(T_b)

    nc.compile()
    return nc


# --------------------------------------------------------------------------
# Entry point
# --------------------------------------------------------------------------
LAST_RESULT = None
LAST_NC = None
LAST_IN_MAPS = None
LAST_META = None


def kernel(x, edge_index, Wg0, Wg1, Wg2, Wfc1, Wfc2):
    from concourse.bass_utils import run_bass_kernel_spmd

    global LAST_RESULT, LAST_NC, LAST_IN_MAPS, LAST_META
    x = np.asarray(x)
    edge_index = np.asarray(edge_index)
    per_core, meta = _preprocess(x, edge_index)
    in_maps = _build_core_inputs(
        x, (np.asarray(Wg0), np.asarray(Wg1), np.asarray(Wg2),
            np.asarray(Wfc1), np.asarray(Wfc2)), per_core, meta)
    nc = _build_bass(meta)
    LAST_NC, LAST_IN_MAPS, LAST_META = nc, in_maps, meta
    res = run_bass_kernel_spmd(nc, in_maps, core_ids=list(range(N_CORES)))
    LAST_RESULT = res
    NL = meta["NL"]
    out = np.concatenate([res.results[c]["out"][:NL] for c in range(N_CORES)],
                         axis=0)
    return out.astype(np.float32)



# revision 17
# speedup vs baseline: 7.2594x; 1.5046x over previous
"""GCN message-passing kernel for 8 Trainium2 NeuronCores.

Model (PyG GCNConv x3 + MLP head + softmax):
    A01 = adjacency + self loops (unit weights), deg = in-degree over A01
    conv(H, W) = D^-1/2 A01 D^-1/2 (H @ W)
    h = x; h = leaky(conv(h, Wg_l)) x3
    y = softmax(leaky(leaky(h @ Wfc1) @ Wfc2))

Key algebraic rewrite: leaky_relu is positively homogeneous, so the
D^-1/2 factors can be pulled out of every leaky() and folded into the
per-layer "message table" G_l:
    G_1 = D^-1/2 (x @ Wg0)
    Ht_{l+1} = leaky(A01 @ G_l)            (pure 0/1 segment-sum!)
    G_{l+1} = D^-1 (Ht_{l+1} @ Wg_l)
    final: z = D^-1/2 leaky(leaky(Ht_4 @ Wfc1) @ Wfc2), out = softmax(z)

Sharding: destination nodes are split into 8 contiguous blocks of 6250
(padded to 6272 = 49 windows of 128). Each layer: every core computes its
G shard (matmul + per-row scale, fused per-window into the previous
layer's scatter so the AllGathers overlap that layer's gather stream),
an AllGather builds the full G table in DRAM, then each core gathers
source rows for its edges with dma_gather(transpose=False) — edge-major:
slot j lands on partition j%128, block j//128, so every 128-slot chunk is
directly the lhsT of the one-hot accumulate  msg^T @ onehot  on the
TensorEngine (no PE transpose, no PSUM staging). int16 indices are sorted
ascending per bucket for DRAM locality; the table is split in two
<32768-row halves; ragged per-window chunk caps = max over cores; gather
groups of 7 windows (~8k idxs) amortize the ~20us per-gather floor under
the ~8144-idx SWDGE ring cap. One-hots are built per window on DVE
(is_equal vs an iota row, small tiles). Self-loop messages skip the
gather entirely: one identity-rhs matmul adds the window's own G rows
from a retained SBUF copy of the local shard (no DRAM readback). The
flush produces the next layer's activations already transposed
(feat x rows), which is exactly the lhsT layout the next matmul needs.
"""

import numpy as np

P = 128
N_CORES = 8


def _gw(NW):
    """Windows per gather group (last group may be ragged).  Large groups
    amortize the ~20us per-gather-instruction floor; the SWDGE ring caps a
    single gather at ~8144 idxs."""
    return 7 if NW % 7 == 0 else 1


# --------------------------------------------------------------------------
# Host-side preprocessing: shard edges by destination, pad to fixed chunk
# counts (SPMD requires an identical instruction stream on all cores).
# --------------------------------------------------------------------------
def _preprocess(x, edge_index):
    N, D = x.shape
    assert D == P
    NL = N // N_CORES                      # real nodes per core
    NW = (NL + P - 1) // P                 # windows per core
    NLP = NW * P                           # padded nodes per core
    NGP = N_CORES * NLP                    # padded global nodes
    # A/B source-table split at the per-core window midpoint: table A holds
    # every core's windows [0, SPLIT_W), table B the rest.  Both stay under
    # the 32768-row int16 gather limit, and the A-half AllGather can fire
    # as soon as each core finishes producing its first SPLIT_W windows,
    # overlapping the B-half production and the B AllGather with the
    # A-half gathers.
    SPLIT_W = (NW + 1) // 2
    SPLIT = SPLIT_W * P

    src = np.asarray(edge_index[0], dtype=np.int64)
    dst = np.asarray(edge_index[1], dtype=np.int64)

    # in-degree INCLUDING the self loop (GCNConv adds A+I); the self-loop
    # message itself is applied on-device via an identity matmul over the
    # local G rows, so loop edges are NOT bucketed.
    deg = (np.bincount(dst, minlength=N) + 1).astype(np.float32)

    sowner = src // NL
    lid_s = src - sowner * NL              # source local id on its core

    owner = dst // NL                      # destination owner core
    lid = dst - owner * NL                 # local dest id on that core
    w = lid // P                           # window
    dr = (lid % P).astype(np.float32)      # one-hot row within window
    half = (lid_s >= SPLIT).astype(np.int64)
    srel = np.where(half == 1,
                    sowner * (NLP - SPLIT) + (lid_s - SPLIT),
                    sowner * SPLIT + lid_s)  # row within table A/B
    assert N_CORES * SPLIT <= 32768 and N_CORES * (NLP - SPLIT) <= 32768

    # bucket key: (core, half, window); sort by srel within each bucket so
    # the gather walks ascending rows (DRAM page locality)
    key = ((owner * 2 + half) * NW + w)
    nbuckets = N_CORES * 2 * NW
    order = np.lexsort((srel, key))
    key_s = key[order]
    srel_s = srel[order]
    dr_s = dr[order]

    counts = np.bincount(key_s, minlength=nbuckets).reshape(N_CORES, 2, NW)
    # ragged per-window chunk caps (max over cores, shared SPMD stream)
    capl = np.ceil(counts[:, 0, :].max(axis=0) / P).astype(np.int64)  # [NW]
    caph = np.ceil(counts[:, 1, :].max(axis=0) / P).astype(np.int64)
    capl = np.maximum(capl, 1)   # keep streams non-empty (small graphs)
    caph = np.maximum(caph, 1)
    CAPW = np.stack([capl, caph], axis=0) * P          # [2, NW] slots
    TL = int(CAPW[0].sum())                            # lo slots per core
    TH = int(CAPW[1].sum())

    # per-(core, half, window) slot base inside that core's lo/hi stream
    wbase = np.zeros((2, NW), dtype=np.int64)
    wbase[0, 1:] = np.cumsum(CAPW[0])[:-1]
    wbase[1, 1:] = np.cumsum(CAPW[1])[:-1]

    start = np.zeros(nbuckets, dtype=np.int64)
    start[1:] = np.cumsum(counts.reshape(-1))[:-1]
    within = np.arange(len(key_s)) - start[key_s]
    h_s = (key_s // NW) % 2
    w_s = key_s % NW
    c_s = key_s // (2 * NW)
    dest = c_s * (TL + TH) + np.where(h_s == 0, 0, TL) \
        + wbase[h_s, w_s] + within

    total_cap = N_CORES * (TL + TH)
    idx_flat = np.zeros(total_cap, dtype=np.int16)
    dr_flat = np.full(total_cap, 200.0, dtype=np.float32)
    idx_flat[dest] = srel_s.astype(np.int16)
    dr_flat[dest] = dr_s

    per_core = []
    for c in range(N_CORES):
        seg_i = idx_flat[c * (TL + TH):(c + 1) * (TL + TH)]
        seg_d = dr_flat[c * (TL + TH):(c + 1) * (TL + TH)]
        ilo, ihi = seg_i[:TL], seg_i[TL:]
        dlo, dhi = seg_d[:TL], seg_d[TL:]
        per_core.append((ilo, ihi, dlo, dhi))

    meta = dict(N=N, NL=NL, NW=NW, NLP=NLP, NGP=NGP, SPLIT_W=SPLIT_W,
                capl=[int(v) for v in capl], caph=[int(v) for v in caph],
                TL=TL, TH=TH, deg=deg)
    return per_core, meta


def _wrap_idx_ragged(flat, gbounds):
    """flat: [T] int16 slot stream (window-major, ragged caps).
    gbounds: slot offsets of each gather-group boundary (len ngrp+1).
    Returns [128, T/16] int16 in dma_gather's wrapped layout: within each
    group block, logical index j lives at [j % 16, j // 16], replicated
    8x across the 128 partitions."""
    cols = []
    for g in range(len(gbounds) - 1):
        block = flat[gbounds[g]:gbounds[g + 1]]
        m = block.reshape(-1, 16).T
        cols.append(np.tile(m, (8, 1)))
    return np.ascontiguousarray(np.concatenate(cols, axis=1))


def _group_bounds(cap, GW):
    """Slot/chunk offsets per gather group given per-window caps."""
    NW = len(cap)
    ngrp = (NW + GW - 1) // GW
    sb = [0]
    kb = [0]
    for g in range(ngrp):
        sb.append(sb[-1] + sum(cap[g * GW:(g + 1) * GW]) * P)
        kb.append(kb[-1] + sum(cap[g * GW:(g + 1) * GW]))
    return sb, kb


def _build_core_inputs(x, Ws, per_core, meta):
    """Build the per-core device input dict."""
    N, NL, NW, NLP = meta["N"], meta["NL"], meta["NW"], meta["NLP"]
    capl, caph = meta["capl"], meta["caph"]
    deg = meta["deg"]
    GW = _gw(NW)
    Wg0, Wg1, Wg2, Wfc1, Wfc2 = Ws

    iota = np.tile(np.arange(P, dtype=np.float32), (P, 1))
    ident = np.eye(P, dtype=np.float32)
    # Wfc2 [256, 2] -> [128, 4]: cols 0:2 first half of u, 2:4 second half
    Wfc2p = np.concatenate([Wfc2[:P, :], Wfc2[P:, :]], axis=1)
    Wfc2p = np.ascontiguousarray(Wfc2p, dtype=np.float32)

    sbl, _ = _group_bounds(capl, GW)
    sbh, _ = _group_bounds(caph, GW)

    def dstrel(dflat, cap):
        # [128, sum(cap)]: col = chunk (window-major), row p = edge slot
        cols = []
        o = 0
        for w_ in range(NW):
            n = cap[w_] * P
            cols.append(dflat[o:o + n].reshape(cap[w_], P).T)
            o += n
        return np.ascontiguousarray(np.concatenate(cols, axis=1))

    in_maps = []
    for c in range(N_CORES):
        ilo, ihi, dlo, dhi = per_core[c]
        xs = np.zeros((NLP, P), dtype=np.float32)
        xs[:NL] = x[c * NL:(c + 1) * NL]
        x_t = np.ascontiguousarray(xs.T)                   # [128, NLP]

        degp = np.ones(NLP, dtype=np.float32)
        degp[:NL] = deg[c * NL:(c + 1) * NL]
        deg_t = np.ascontiguousarray(degp.reshape(NW, P).T)  # [128, NW]

        in_maps.append({
            "x_t": x_t,
            "deg_t": deg_t,
            "idx_lo": _wrap_idx_ragged(ilo, sbl),
            "dstrel_lo": dstrel(dlo, capl),
            "idx_hi": _wrap_idx_ragged(ihi, sbh),
            "dstrel_hi": dstrel(dhi, caph),
            "iota": iota,
            "ident": ident,
            "Wg0": np.ascontiguousarray(Wg0, dtype=np.float32),
            "Wg1": np.ascontiguousarray(Wg1, dtype=np.float32),
            "Wg2": np.ascontiguousarray(Wg2, dtype=np.float32),
            "Wfc1": np.ascontiguousarray(Wfc1, dtype=np.float32),
            "Wfc2p": Wfc2p,
        })
    return in_maps


# --------------------------------------------------------------------------
# Device program
# --------------------------------------------------------------------------
def _build_bass(meta, mock_cc=False, opts=None, reps=1):
    opts = opts or {}
    from concourse import bass, bacc, mybir
    import concourse.tile as tile

    NW, NLP, NGP = meta["NW"], meta["NLP"], meta["NGP"]
    SPLIT_W = meta["SPLIT_W"]
    SPLIT = SPLIT_W * P
    NBW = NW - SPLIT_W                     # B-half windows (may be 0)
    NA = N_CORES * SPLIT                   # table A rows
    NB = N_CORES * (NLP - SPLIT)           # table B rows
    capl, caph = meta["capl"], meta["caph"]
    TL, TH = meta["TL"], meta["TH"]
    GW = _gw(NW)
    NGRP = (NW + GW - 1) // GW
    sbl, kbl = _group_bounds(capl, GW)     # slot / chunk offsets per group
    sbh, kbh = _group_bounds(caph, GW)
    ckl = np.concatenate([[0], np.cumsum(capl)]).astype(int)  # per window
    ckh = np.concatenate([[0], np.cumsum(caph)]).astype(int)
    GLMAXL = max(sbl[g + 1] - sbl[g] for g in range(NGRP))
    GLMAXH = max(sbh[g + 1] - sbh[g] for g in range(NGRP))
    KMAXL = max(kbl[g + 1] - kbl[g] for g in range(NGRP))
    KMAXH = max(kbh[g + 1] - kbh[g] for g in range(NGRP))
    f32 = mybir.dt.float32
    bf16 = mybir.dt.bfloat16
    i16 = mybir.dt.int16
    ALL = [list(range(N_CORES))]

    nc = bacc.Bacc("TRN2", target_bir_lowering=False, debug=False,
                   num_devices=N_CORES)

    x_t_d = nc.dram_tensor("x_t", [P, NLP], f32, kind="ExternalInput")
    deg_d = nc.dram_tensor("deg_t", [P, NW], f32, kind="ExternalInput")
    ilo_d = nc.dram_tensor("idx_lo", [P, TL // 16], i16, kind="ExternalInput")
    drl_d = nc.dram_tensor("dstrel_lo", [P, TL // P], f32,
                           kind="ExternalInput")
    ihi_d = nc.dram_tensor("idx_hi", [P, TH // 16], i16, kind="ExternalInput")
    drh_d = nc.dram_tensor("dstrel_hi", [P, TH // P], f32,
                           kind="ExternalInput")
    iota_d = nc.dram_tensor("iota", [P, P], f32, kind="ExternalInput")
    ident_d = nc.dram_tensor("ident", [P, P], f32, kind="ExternalInput")
    wg_d = [nc.dram_tensor(f"Wg{i}", [P, P], f32, kind="ExternalInput")
            for i in range(3)]
    wfc1_d = nc.dram_tensor("Wfc1", [P, 256], f32, kind="ExternalInput")
    wfc2_d = nc.dram_tensor("Wfc2p", [P, 4], f32, kind="ExternalInput")
    out_d = nc.dram_tensor("out", [NLP, 2], f32, kind="ExternalOutput")

    with tile.TileContext(nc) as tc:
        with (
            tc.tile_pool(name="const", bufs=1) as cpool,
            tc.tile_pool(name="msg", bufs=3) as mpool,
            tc.tile_pool(name="oh", bufs=3) as ohpool,
            tc.tile_pool(name="work", bufs=3) as wpool,
            tc.tile_pool(name="acc", bufs=3, space="PSUM") as ppool,
            tc.tile_pool(name="accy", bufs=1, space="PSUM") as p2pool,
            tc.tile_pool(name="dram", bufs=1, space="DRAM") as dpool,
        ):
            # ---- constants / casts ----
            T_a = cpool.tile([P, NLP], bf16, name="T_a")
            nc.gpsimd.dma_start(out=T_a[:], in_=x_t_d[:])   # f32->bf16 cast
            T_b = cpool.tile([P, NLP], bf16, name="T_b")

            iota_sb = cpool.tile([P, P], bf16, name="iota_sb")
            nc.gpsimd.dma_start(out=iota_sb[:], in_=iota_d[:])
            ident_sb = cpool.tile([P, P], bf16, name="ident_sb")
            nc.gpsimd.dma_start(out=ident_sb[:], in_=ident_d[:])
            wg_sb = []
            for i in range(3):
                t = cpool.tile([P, P], bf16, name=f"wg_sb{i}")
                nc.gpsimd.dma_start(out=t[:], in_=wg_d[i][:])
                wg_sb.append(t)
            wfc1_sb = cpool.tile([P, 256], bf16, name="wfc1_sb")
            nc.gpsimd.dma_start(out=wfc1_sb[:], in_=wfc1_d[:])
            wfc2_sb = cpool.tile([P, 4], bf16, name="wfc2_sb")
            nc.gpsimd.dma_start(out=wfc2_sb[:], in_=wfc2_d[:])
            drl_sb = cpool.tile([P, TL // P], bf16, name="drl_sb")
            nc.gpsimd.dma_start(out=drl_sb[:], in_=drl_d[:])  # f32->bf16
            ilo_sb = cpool.tile([P, TL // 16], i16, name="ilo_sb")
            nc.sync.dma_start(out=ilo_sb[:], in_=ilo_d[:])
            drh_sb = cpool.tile([P, TH // P], bf16, name="drh_sb")
            nc.gpsimd.dma_start(out=drh_sb[:], in_=drh_d[:])
            ihi_sb = cpool.tile([P, TH // 16], i16, name="ihi_sb")
            nc.sync.dma_start(out=ihi_sb[:], in_=ihi_d[:])

            deg_sb = cpool.tile([P, NW], f32, name="deg_sb")
            nc.sync.dma_start(out=deg_sb[:], in_=deg_d[:])
            invdeg = cpool.tile([P, NW], f32, name="invdeg")
            nc.vector.reciprocal(invdeg[:], deg_sb[:])
            dinv = cpool.tile([P, NW], f32, name="dinv")
            nc.scalar.sqrt(dinv[:], invdeg[:])

            # NB: collective outputs in Local addr space — Shared
            # scratchpad DMA reads measured ~3x slower on the gather path.
            # Separate A/B local shards + tables give the Tile scheduler
            # precise deps: AG_A fires after the first SPLIT_W windows of
            # production and overlaps the rest; AG_B overlaps the A-half
            # gathers of the scatter phase.
            gfA = [dpool.tile([NA, P], bf16, name=f"gfA{i}") for i in range(3)]
            gfB = ([dpool.tile([NB, P], bf16, name=f"gfB{i}") for i in range(3)]
                   if NBW else None)
            glocA = [dpool.tile([SPLIT, P], bf16, name=f"glocA{i}")
                     for i in range(3)]
            glocB = ([dpool.tile([NLP - SPLIT, P], bf16, name=f"glocB{i}")
                      for i in range(3)] if NBW else None)

            Copy = mybir.ActivationFunctionType.Copy

            # retained node-major G tables (self-loop rows; double-buffered
            # across layers) — replaces per-window DRAM readbacks
            gwkeep = [cpool.tile([P, NLP], bf16, name=f"gwkeep{i}")
                      for i in range(2)]

            def prod_window(l, Tsrc, w):
                """Produce G_l window w (node-major) from activations Tsrc:
                matmul + per-row scale into the retained SBUF table, DMA the
                shard row-block out, and fire the A/B AllGathers at the split
                points.  Called fused from the previous layer's scatter so
                the collectives overlap the tail of that layer's gathers."""
                scale = dinv if l == 0 else invdeg
                ps = ppool.tile([P, P], f32, tag="acc", name="psg")
                nc.tensor.matmul(ps[:], lhsT=Tsrc[:, w * P:(w + 1) * P],
                                 rhs=wg_sb[l][:], start=True, stop=True)
                gk = gwkeep[l % 2]
                nc.scalar.activation(gk[:, w * P:(w + 1) * P], ps[:], Copy,
                                     bias=0.0, scale=scale[:, w:w + 1])
                if w < SPLIT_W:
                    nc.sync.dma_start(out=glocA[l][w * P:(w + 1) * P, :],
                                      in_=gk[:, w * P:(w + 1) * P])
                else:
                    wb = w - SPLIT_W
                    nc.sync.dma_start(out=glocB[l][wb * P:(wb + 1) * P, :],
                                      in_=gk[:, w * P:(w + 1) * P])
                if w == SPLIT_W - 1:
                    if mock_cc:
                        for c in range(N_CORES):
                            nc.sync.dma_start(
                                out=gfA[l][c * SPLIT:(c + 1) * SPLIT, :],
                                in_=glocA[l][:])
                    else:
                        nc.gpsimd.collective_compute(
                            "AllGather", mybir.AluOpType.bypass,
                            replica_groups=ALL,
                            ins=[glocA[l][:]], outs=[gfA[l][:]])
                if w == NW - 1 and NBW:
                    if mock_cc:
                        nsh = NLP - SPLIT
                        for c in range(N_CORES):
                            nc.sync.dma_start(
                                out=gfB[l][c * nsh:(c + 1) * nsh, :],
                                in_=glocB[l][:])
                    else:
                        nc.gpsimd.collective_compute(
                            "AllGather", mybir.AluOpType.bypass,
                            replica_groups=ALL,
                            ins=[glocB[l][:]], outs=[gfB[l][:]])

            def leaky_into(dst_ap, ps):
                t = wpool.tile([P, dst_ap.shape[-1]], f32, tag="lk", name="lkt")
                nc.scalar.activation(t[:], ps[:], Copy, bias=0.0, scale=0.01)
                nc.vector.tensor_tensor(out=dst_ap, in0=ps[:], in1=t[:],
                                        op=mybir.AluOpType.max)


            def scatter_tr(l, Tdst, last=False):
                """Transpose-mode gather (dma_gather(transpose=True)) returns
                messages feature-major [128f, n_idxs]; each 128-edge chunk is
                PE-transposed back to edge-major via the identity trick,
                staged through PSUM(bf16) -> SBUF (Act copy), then accumulated
                with the one-hot matmul.  Chunk counts are ragged per window;
                the self-loop message is injected exactly via an identity-rhs
                matmul over the retained SBUF G rows (no gather slots, no
                DRAM readback).  When not `last`, the next layer's production
                for each window is emitted right after its leaky, so the
                AllGathers fire while this layer's gathers still stream."""
                glo_ap = gfA[l][:]
                # degenerate graphs (single window) still issue B gathers
                # with idx 0 and all-dead one-hot columns
                ghi_ap = gfB[l][:] if NBW else gfA[l][0:P, :]
                gk = gwkeep[l % 2]

                # The Pool engine runs gathers in emission order, and every
                # B-table gather waits on AG_B.  Emit lo a group ahead of hi
                # so A-side gathers keep streaming while AG_B is in flight.
                mtiles = {}

                def issue_lo(g):
                    if g >= NGRP:
                        return
                    GLLg = sbl[g + 1] - sbl[g]
                    t = mpool.tile([P, GLMAXL], bf16, tag="mlo", name="mlo")
                    if not opts.get("skip_gather"):
                        # edge-major gather: slot j lands on partition j%128,
                        # block j//128 — each 128-slot chunk is directly the
                        # lhsT of the one-hot accumulate (no PE transpose)
                        nc.gpsimd.dma_gather(
                            out_ap=t[:, 0:GLLg].rearrange("p (c e) -> p c e",
                                                          e=P),
                            in_ap=glo_ap,
                            idxs_ap=ilo_sb[:, sbl[g] // 16:sbl[g + 1] // 16],
                            num_idxs=GLLg, num_idxs_reg=GLLg, elem_size=P,
                            transpose=False, single_packet=False)
                    mtiles[("lo", g)] = t

                def issue_hi(g):
                    if g >= NGRP:
                        return
                    GLHg = sbh[g + 1] - sbh[g]
                    t = mpool.tile([P, GLMAXH], bf16, tag="mhi", name="mhi")
                    if not opts.get("skip_gather"):
                        nc.gpsimd.dma_gather(
                            out_ap=t[:, 0:GLHg].rearrange("p (c e) -> p c e",
                                                          e=P),
                            in_ap=ghi_ap,
                            idxs_ap=ihi_sb[:, sbh[g] // 16:sbh[g + 1] // 16],
                            num_idxs=GLHg, num_idxs_reg=GLHg, elem_size=P,
                            transpose=False, single_packet=False)
                    mtiles[("hi", g)] = t

                issue_lo(0)
                issue_lo(1)
                issue_lo(2)
                issue_hi(0)
                issue_hi(1)
                OHL = max(capl) * P
                OHH = max(caph) * P
                for g in range(NGRP):
                    mlo = mtiles.pop(("lo", g))
                    mhi = mtiles.pop(("hi", g))
                    if opts.get("skip_matmul"):
                        # timing-only variant: consume gathers/onehots with
                        # cheap DVE reduces so nothing dead-codes
                        X_ = mybir.AxisListType.X
                        tiles = []
                        if not opts.get("skip_gather"):
                            tiles += [mlo[:, 0:P], mhi[:, 0:P]]
                        if not opts.get("skip_onehot"):
                            for w in range(g * GW, min((g + 1) * GW, NW)):
                                ohw = ohpool.tile([P, OHL], bf16, tag="ohlo",
                                                  name="ohw_lo")
                                nc.vector.tensor_tensor(
                                    out=ohw[:, 0:capl[w] * P].rearrange(
                                        "p (k r) -> p k r", r=P),
                                    in0=iota_sb[:].unsqueeze(1)
                                        .to_broadcast([P, capl[w], P]),
                                    in1=drl_sb[:, ckl[w]:ckl[w + 1]]
                                        .unsqueeze(2)
                                        .to_broadcast([P, capl[w], P]),
                                    op=mybir.AluOpType.is_equal)
                                tiles.append(ohw[:, 0:capl[w] * P])
                        for t_ in tiles:
                            rr = wpool.tile([P, 1], f32, tag="rd", name="rd")
                            nc.vector.reduce_sum(out=rr[:], in_=t_,
                                                 axis=X_)
                            nc.vector.tensor_tensor(
                                out=Tdst[:, g:g + 1], in0=rr[:],
                                in1=rr[:], op=mybir.AluOpType.add)
                        issue_lo(g + 3)
                        issue_hi(g + 2)
                        continue
                    for w in range(g * GW, min((g + 1) * GW, NW)):
                        nl = capl[w]
                        nh = caph[w]
                        ntot = nl + nh
                        lo_off = ckl[w] - kbl[g]   # chunk offset in group
                        hi_off = ckh[w] - kbh[g]

                        # per-window one-hots (small tiles keep SBUF free
                        # for deep gather buffering)
                        ohw_lo = ohpool.tile([P, OHL], bf16, tag="ohlo",
                                             name="ohw_lo")
                        ohw_hi = ohpool.tile([P, OHH], bf16, tag="ohhi",
                                             name="ohw_hi")
                        nc.vector.tensor_tensor(
                            out=ohw_lo[:, 0:nl * P].rearrange(
                                "p (k r) -> p k r", r=P),
                            in0=iota_sb[:].unsqueeze(1)
                                .to_broadcast([P, nl, P]),
                            in1=drl_sb[:, ckl[w]:ckl[w + 1]].unsqueeze(2)
                                .to_broadcast([P, nl, P]),
                            op=mybir.AluOpType.is_equal)
                        nc.vector.tensor_tensor(
                            out=ohw_hi[:, 0:nh * P].rearrange(
                                "p (k r) -> p k r", r=P),
                            in0=iota_sb[:].unsqueeze(1)
                                .to_broadcast([P, nh, P]),
                            in1=drh_sb[:, ckh[w]:ckh[w + 1]].unsqueeze(2)
                                .to_broadcast([P, nh, P]),
                            op=mybir.AluOpType.is_equal)

                        def chunk_aps(k, nl=nl, lo_off=lo_off,
                                      hi_off=hi_off, mlo=mlo, mhi=mhi,
                                      ohw_lo=ohw_lo, ohw_hi=ohw_hi):
                            if k < nl:
                                c = lo_off + k
                                return (mlo[:, c * P:(c + 1) * P],
                                        ohw_lo[:, k * P:(k + 1) * P])
                            c = hi_off + (k - nl)
                            return (mhi[:, c * P:(c + 1) * P],
                                    ohw_hi[:, (k - nl) * P:(k - nl + 1) * P])

                        # edge-major chunks feed the accumulate matmul
                        # directly: ps[f, r] += sum_e msg[e, f] * oh[e, r]
                        ps = ppool.tile([P, P], f32, tag="acc", name="pss")
                        for k in range(ntot):
                            m_ap, oh_ap = chunk_aps(k)
                            if opts.get("fixed_msg"):
                                m_ap = ohw_lo[:, 0:P]
                            nc.tensor.matmul(ps[:], lhsT=m_ap,
                                             rhs=oh_ap,
                                             start=(k == 0), stop=False)
                        # self-loop: ps[f, r] += G_l[w*128+r, f] from the
                        # retained SBUF table (no DRAM readback)
                        nc.tensor.matmul(ps[:], lhsT=gk[:, w * P:(w + 1) * P],
                                         rhs=ident_sb[:],
                                         start=False, stop=True)
                        leaky_into(Tdst[:, w * P:(w + 1) * P], ps)
                        if not last:
                            prod_window(l + 1, Tdst, w)
                    issue_lo(g + 3)
                    issue_hi(g + 2)

            def head(Tsrc):
                X = mybir.AxisListType.X
                Exp = mybir.ActivationFunctionType.Exp
                for w in range(NW):
                    y1t = []
                    for h in range(2):
                        ps1 = ppool.tile([P, P], f32, tag="acc", name="ps1")
                        nc.tensor.matmul(ps1[:],
                                         lhsT=wfc1_sb[:, h * P:(h + 1) * P],
                                         rhs=Tsrc[:, w * P:(w + 1) * P],
                                         start=True, stop=True)
                        yt = wpool.tile([P, P], bf16, tag=f"y1_{h}",
                                        name="yt")
                        leaky_into(yt[:], ps1)
                        y1t.append(yt)
                    ps2 = p2pool.tile([P, 2], f32, tag="y2", name="ps2")
                    nc.tensor.matmul(ps2[:], lhsT=y1t[0][:],
                                     rhs=wfc2_sb[:, 0:2],
                                     start=True, stop=False)
                    nc.tensor.matmul(ps2[:], lhsT=y1t[1][:],
                                     rhs=wfc2_sb[:, 2:4],
                                     start=False, stop=True)
                    y2 = wpool.tile([P, 2], f32, tag="y2s", name="y2")
                    leaky_into(y2[:], ps2)
                    z = wpool.tile([P, 2], f32, tag="z", name="z")
                    nc.scalar.activation(z[:], y2[:], Copy, bias=0.0,
                                         scale=dinv[:, w:w + 1])
                    negm = wpool.tile([P, 1], f32, tag="m", name="negm")
                    nc.vector.reduce_max(out=negm[:], in_=z[:], axis=X,
                                         negate=True)
                    e = wpool.tile([P, 2], f32, tag="e", name="e")
                    nc.scalar.activation(e[:], z[:], Exp,
                                         bias=negm[:, 0:1], scale=1.0)
                    s = wpool.tile([P, 1], f32, tag="s", name="s")
                    nc.vector.reduce_sum(out=s[:], in_=e[:], axis=X)
                    rs = wpool.tile([P, 1], f32, tag="rs", name="rs")
                    nc.vector.reciprocal(rs[:], s[:])
                    o = wpool.tile([P, 2], f32, tag="o", name="o")
                    nc.vector.tensor_scalar(out=o[:], in0=e[:],
                                            scalar1=rs[:, 0:1], scalar2=None,
                                            op0=mybir.AluOpType.mult)
                    nc.sync.dma_start(out=out_d[w * P:(w + 1) * P, :],
                                      in_=o[:])

            sc = scatter_tr
            for _rep in range(reps):
                if opts.get("skip_scatter"):
                    for l in range(3):
                        for w in range(NW):
                            prod_window(l, T_a, w)
                    head(T_a)
                else:
                    for w in range(NW):
                        prod_window(0, T_a, w)
                    sc(0, T_b)             # fused: also produces G_1
                    sc(1, T_a)             # fused: also produces G_2
                    sc(2, T_b, last=True)
                    head(T_b)

    nc.compile()
    return nc


# --------------------------------------------------------------------------
# Entry point
# --------------------------------------------------------------------------
LAST_RESULT = None
LAST_NC = None
LAST_IN_MAPS = None
LAST_META = None


def kernel(x, edge_index, Wg0, Wg1, Wg2, Wfc1, Wfc2):
    from concourse.bass_utils import run_bass_kernel_spmd

    global LAST_RESULT, LAST_NC, LAST_IN_MAPS, LAST_META
    x = np.asarray(x)
    edge_index = np.asarray(edge_index)
    per_core, meta = _preprocess(x, edge_index)
    in_maps = _build_core_inputs(
        x, (np.asarray(Wg0), np.asarray(Wg1), np.asarray(Wg2),
            np.asarray(Wfc1), np.asarray(Wfc2)), per_core, meta)
    nc = _build_bass(meta)
    LAST_NC, LAST_IN_MAPS, LAST_META = nc, in_maps, meta
    res = run_bass_kernel_spmd(nc, in_maps, core_ids=list(range(N_CORES)))
    LAST_RESULT = res
    NL = meta["NL"]
    out = np.concatenate([res.results[c]["out"][:NL] for c in range(N_CORES)],
                         axis=0)
    return out.astype(np.float32)

